# revision 25
# baseline (speedup 1.0000x reference)
"""AdaptiveBlock Trainium2 kernel, 8-core data-parallel.

Reference computation (per batch b):
    y      = mean(x[b], axis=(H, W))                    # (C,)
    h      = gelu(y @ w1.T)                             # (HIDDEN,)
    y'     = gelu(h @ w2.T)                             # (C,)
    A      = (y' @ wA.T).reshape(H, R)
    Bm     = (y' @ wB.T).reshape(R, W)
    attn   = sigmoid(A @ Bm)                            # (H, W)
    out[b] = broadcast attn over C                      # (C, H, W)

Sharding: B=32 split over 8 cores (4 batches/core), weights replicated.
Memory-bound: each core reads 12.8 MB of x and writes 12.8 MB of output;
HBM roofline ~72 us/core.

Device layout / scheduling notes:
- y is computed transposed (C on partitions) directly by free-dim reduces.
- All weights are passed pre-transposed AND pre-cast to bf16 host-side
  (contraction dim on partitions); matmul operands are bf16, PSUM f32.
  The f32 sums feeding the MLP are cast to bf16 on the ScalarEngine.
  All activations here are ~1e-2 magnitude and the output is sigmoid
  (~0.5 +- 1e-4), so bf16 operand noise is ~1e-6 relative on the output.
- wA additionally has its output columns permuted host-side from i*8+r to
  r*56+i so the per-batch (8, 56) bilinear lhsT is a contiguous-row
  SBUF->SBUF DMA.
- The (56, 56) attention map is flattened to (1, 3136) bf16, broadcast to
  128 partitions by a K=1 ones-matmul on the TensorEngine, with sigmoid
  fused into the PSUM->SBUF copy on the ScalarEngine.
- The 4 batches are processed in 2 groups of 2 so group 0's output DMAs
  overlap group 1's input DMAs. Inputs ride the SWDGE (gpsimd) queue,
  outputs + weights the SP HWDGE queue, small SBUF->SBUF moves the ACT
  HWDGE queue - three independent DMA dispatch FIFOs.
"""

import numpy as np
import ml_dtypes

import concourse.bass as bass
import concourse.tile as tile
from concourse import bacc, mybir
from concourse.bass_utils import run_bass_kernel_spmd

F32 = mybir.dt.float32
BF16 = mybir.dt.bfloat16

B, C, H, W = 32, 256, 56, 56
HW = H * W                      # 3136
HIDDEN = 512
RANK = 8
HR = H * RANK                   # 448
NCORES = 8
BLOC = B // NCORES              # 4 batches per core
P = 128
NCC = C // P                    # 2 channel chunks
NHH = HIDDEN // P               # 4 hidden chunks
BCHUNK = 512                    # matmul moving free-dim max (PSUM bank)
NGRP = 2                        # batch groups per core
GB = BLOC // NGRP               # batches per group


def build_bass(sim_compat: bool = False) -> bacc.Bacc:
    """sim_compat=True swaps exact Gelu (not implemented in CoreSim) for a
    0.5*x stand-in; with |gelu-input| ~ 0.02 this perturbs the final sigmoid
    output by ~1e-5 relative, so the sim still validates all layout/dataflow.
    Hardware builds always use the exact erf-based Gelu."""
    gelu_f = (
        mybir.ActivationFunctionType.Copy
        if sim_compat
        else mybir.ActivationFunctionType.Gelu
    )
    gelu_s = 0.5 if sim_compat else 1.0
    nc = bacc.Bacc(num_devices=NCORES)

    HRP = 2 * C                                        # 512: r*64+i padded cols
    WPACK = NCC * HIDDEN + NHH * C + 2 * NCC * HRP     # 4096 bf16 columns
    x_d = nc.dram_tensor("x", [BLOC, C, HW], F32, kind="ExternalInput")
    wpk_d = nc.dram_tensor("wpk", [P, WPACK], BF16, kind="ExternalInput")
    out_d = nc.dram_tensor("out", [BLOC, C, HW], F32, kind="ExternalOutput")

    x_v = x_d.ap().rearrange("b (cc p) hw -> b cc p hw", p=P)
    x_vb = x_d.ap().rearrange("b (cc p) hw -> b p cc hw", p=P)
    out_v = out_d.ap().rearrange("b (cc p) hw -> b cc p hw", p=P)
    OF_W1, OF_W2, OF_WA, OF_WB = 0, NCC * HIDDEN, NCC * HIDDEN + NHH * C, NCC * HIDDEN + NHH * C + NCC * HRP

    nchunks = (HW + BCHUNK - 1) // BCHUNK

    with tile.TileContext(nc) as tc:
        with (
            tc.tile_pool(name="xin", bufs=8) as xpool,
            tc.tile_pool(name="persist", bufs=1) as ppool,
            tc.tile_pool(name="small", bufs=2) as spool,
            tc.tile_pool(name="bc", bufs=4) as bcpool,
            tc.tile_pool(name="ps_small", bufs=2, space="PSUM") as ps_small,
            tc.tile_pool(name="ps_ab", bufs=1, space="PSUM") as ps_ab,
            tc.tile_pool(name="ps_bc", bufs=4, space="PSUM") as ps_bc,
        ):
            # --- single packed weight load; queued in the SWDGE FIFO after
            # the first 4 x chunks (weights are first needed ~6 chunks in,
            # and x chunk 1 gates the whole reduce pipeline)
            wpk = ppool.tile([P, WPACK], BF16, tag="wpk", name="wpk")
            w1t = [wpk[:, OF_W1 + cc * HIDDEN : OF_W1 + (cc + 1) * HIDDEN]
                   for cc in range(NCC)]
            w2t = [wpk[:, OF_W2 + hh * C : OF_W2 + (hh + 1) * C]
                   for hh in range(NHH)]
            wat = [wpk[:, OF_WA + cc * HRP : OF_WA + (cc + 1) * HRP]
                   for cc in range(NCC)]
            wbt = [wpk[:, OF_WB + cc * HRP : OF_WB + (cc + 1) * HRP]
                   for cc in range(NCC)]
            ones = ppool.tile([1, P], BF16, tag="ones", name="ones")
            nc.vector.memset(ones[:], 1.0)

            # --- stream ALL x in up front; spatial-sum per (b, chunk).
            # Group 0's cc=1 sums run on the ScalarEngine (activation
            # accum_out) so the DVE reduce chain isn't the serial
            # bottleneck; everything later in the ACT FIFO belongs to
            # group 0's compute chain, so ACT never head-of-line blocks
            # group 1 (whose sums all run on DVE).
            ysum = [[ppool.tile([P, GB], F32, tag=f"ysum{g}{cc}",
                                name=f"ysum{g}{cc}") for cc in range(NCC)]
                    for g in range(NGRP)]

            def spatial_sum(view, g, cc, j):
                # group 0's cc=1 sums run on ACT (accum_out) in parallel
                # with DVE; everything else on DVE
                if g == 0 and cc == 1:
                    nc.scalar.activation(
                        view, view, mybir.ActivationFunctionType.Copy,
                        accum_out=ysum[g][cc][:, j : j + 1],
                    )
                else:
                    nc.vector.reduce_sum(
                        ysum[g][cc][:, j : j + 1], view,
                        axis=mybir.AxisListType.X,
                    )

            hparts = ppool.tile([P, 2], F32, tag="hparts", name="hparts")
            xt_b2c1 = None
            for b in range(BLOC):
                g, j = divmod(b, GB)
                for cc in range(NCC):
                    xt = xpool.tile([P, HW], F32, tag="xt", name="xt")
                    if (b, cc) == (2, 1):
                        # sum deferred to ACT's idle window after g0's MLP
                        nc.gpsimd.dma_start(xt[:], x_v[b, cc])
                        xt_b2c1 = xt
                        continue
                    if b == BLOC - 1 and cc == NCC - 1:
                        # split the last chunk so its reduce tail is half
                        # as long after the stream ends
                        HH = HW // 2
                        nc.gpsimd.dma_start(xt[:, 0:HH], x_v[b, cc][:, 0:HH])
                        nc.vector.reduce_sum(hparts[:, 0:1], xt[:, 0:HH],
                                             axis=mybir.AxisListType.X)
                        nc.gpsimd.dma_start(xt[:, HH:HW], x_v[b, cc][:, HH:HW])
                        nc.vector.reduce_sum(hparts[:, 1:2], xt[:, HH:HW],
                                             axis=mybir.AxisListType.X)
                        nc.vector.tensor_add(
                            ysum[g][cc][:, j : j + 1], hparts[:, 0:1],
                            hparts[:, 1:2],
                        )
                    else:
                        nc.gpsimd.dma_start(xt[:], x_v[b, cc])
                        spatial_sum(xt[:], g, cc, j)
                    if (b, cc) == (1, 1):
                        nc.gpsimd.dma_start(wpk[:], wpk_d.ap())

            def make_ysb(g):
                ysb = [ppool.tile([P, GB], BF16, tag=f"ysb{g}{cc}",
                                  name=f"ysb{g}{cc}") for cc in range(NCC)]
                for cc in range(NCC):
                    if g == 0:
                        nc.scalar.copy(ysb[cc][:], ysum[g][cc][:])
                    else:
                        nc.vector.tensor_copy(ysb[cc][:], ysum[g][cc][:])
                return ysb

            def mlp_group(g, ysb):
                """MLP + A|B matmuls for one batch group -> ab_sb tile."""
                hT = [ppool.tile([P, GB], BF16, tag=f"hT{g}{hh}",
                                 name=f"hT{g}{hh}") for hh in range(NHH)]
                for hh in range(NHH):
                    ph = ps_small.tile([P, GB], F32, tag="ps", name="ps")
                    for cc in range(NCC):
                        nc.tensor.matmul(
                            ph[:], w1t[cc][:, hh * P : (hh + 1) * P], ysb[cc][:],
                            start=(cc == 0), stop=(cc == NCC - 1),
                        )
                    nc.scalar.activation(hT[hh][:], ph[:], gelu_f,
                                         scale=gelu_s / HW)
                ypT = [ppool.tile([P, GB], BF16, tag=f"ypT{g}{cc}",
                                  name=f"ypT{g}{cc}") for cc in range(NCC)]
                for cc in range(NCC):
                    pyp = ps_small.tile([P, GB], F32, tag="ps", name="ps")
                    for hh in range(NHH):
                        nc.tensor.matmul(
                            pyp[:], w2t[hh][:, cc * P : (cc + 1) * P], hT[hh][:],
                            start=(hh == 0), stop=(hh == NHH - 1),
                        )
                    nc.scalar.activation(ypT[cc][:], pyp[:], gelu_f, scale=gelu_s)
                pab = ps_ab.tile([GB, 2 * BCHUNK], F32, tag="pab", name="pab")
                for half, wt in ((0, wat), (1, wbt)):
                    for cc in range(NCC):
                        nc.tensor.matmul(
                            pab[:, half * BCHUNK : (half + 1) * BCHUNK],
                            ypT[cc][:], wt[cc][:],
                            start=(cc == 0), stop=(cc == NCC - 1),
                        )
                ab_sb = ppool.tile([GB, 2 * BCHUNK], BF16, tag=f"ab{g}",
                                   name=f"ab{g}")
                nc.scalar.copy(ab_sb[:], pab[:])
                return ab_sb

            def bilinear_flat(ab_sb, j):
                """Per batch: t = tanh((A @ B)/2) flattened to (1, HW) bf16."""
                ar = spool.tile([RANK, 64], BF16, tag="ar", name="ar")
                br = spool.tile([RANK, 64], BF16, tag="br", name="br")
                nc.sync.dma_start(
                    ar[:],
                    ab_sb[j : j + 1, 0:BCHUNK].rearrange(
                        "o (r i) -> o r i", r=RANK),
                )
                nc.sync.dma_start(
                    br[:],
                    ab_sb[j : j + 1, BCHUNK : 2 * BCHUNK].rearrange(
                        "o (r i) -> o r i", r=RANK),
                )
                pm = ps_small.tile([H, W], F32, tag="ps", name="ps")
                nc.tensor.matmul(pm[:], ar[:, 0:H], br[:, 0:W],
                                 start=True, stop=True)
                msb = spool.tile([H, W], BF16, tag="msb", name="msb")
                nc.scalar.activation(msb[:], pm[:],
                                     mybir.ActivationFunctionType.Tanh,
                                     scale=0.5)
                flat = spool.tile([1, HW], BF16, tag="flat", name="flat")
                nc.sync.dma_start(
                    flat[0:1, :].rearrange("o (i j) -> o i j", i=H), msb[:, :]
                )
                return flat

            def bcast_out(b, flat):
                """Broadcast t to 128 partitions, apply 0.5*t+0.5, store."""
                bc = bcpool.tile([P, HW], F32, tag="bc", name="bc")
                for k in range(nchunks):
                    off = k * BCHUNK
                    sz = min(BCHUNK, HW - off)
                    pb = ps_bc.tile([P, sz], F32, tag="psbc", name="psbc")
                    nc.tensor.matmul(
                        pb[:], ones[:], flat[0:1, off : off + sz],
                        start=True, stop=True,
                    )
                    nc.scalar.activation(
                        bc[:, off : off + sz], pb[:],
                        mybir.ActivationFunctionType.Copy,
                        bias=0.5, scale=0.5,
                    )
                for cc in range(NCC):
                    nc.sync.dma_start(out_v[b, cc], bc[:])

            # Emission (~execution) order: interleave group 1's MLP between
            # batch 0's and batch 1's broadcast sections so the PE FIFO
            # doesn't serialize g1's chain behind both g0 broadcasts.
            ysb0 = make_ysb(0)
            ab0 = mlp_group(0, ysb0)
            nc.scalar.activation(
                xt_b2c1[:], xt_b2c1[:], mybir.ActivationFunctionType.Copy,
                accum_out=ysum[1][1][:, 0:1],
            )
            flat0 = bilinear_flat(ab0, 0)
            flat1 = bilinear_flat(ab0, 1)
            bcast_out(0, flat0)
            bcast_out(1, flat1)
            ysb1 = make_ysb(1)
            ab1 = mlp_group(1, ysb1)
            flat2 = bilinear_flat(ab1, 0)
            flat3 = bilinear_flat(ab1, 1)
            bcast_out(2, flat2)
            bcast_out(3, flat3)

    nc.compile()
    return nc


def _prep_in_maps(x, w1, w2, wA, wB):
    x = np.ascontiguousarray(np.asarray(x, dtype=np.float32))
    w1 = np.asarray(w1, dtype=np.float32)
    w2 = np.asarray(w2, dtype=np.float32)
    wA = np.asarray(wA, dtype=np.float32)
    wB = np.asarray(wB, dtype=np.float32)

    bf = ml_dtypes.bfloat16
    w1t = np.ascontiguousarray(w1.T)                       # (C, HIDDEN)
    w2t = np.ascontiguousarray(w2.T)                       # (HIDDEN, C)
    # permute wA rows i*8+r -> r*64+i (8 zero pad cols per r) and wB rows
    # r*56+j -> r*64+j, then transpose: 64-aligned r-chunks make the
    # per-batch (8, 56) bilinear operand extraction a clean strided DMA
    HRP = 2 * C
    wap = np.zeros((RANK, 64, C), np.float32)
    wap[:, :H, :] = wA.reshape(H, RANK, C).transpose(1, 0, 2)
    wat = np.ascontiguousarray(wap.reshape(HRP, C).T)
    wbp = np.zeros((RANK, 64, C), np.float32)
    wbp[:, :W, :] = wB.reshape(RANK, W, C)
    wbt = np.ascontiguousarray(wbp.reshape(HRP, C).T)

    # pack per-partition: [w1t cc-chunks | w2t hh-chunks | wat | wbt]
    def chunked(m, n):          # (n*128, F) -> (128, n*F), chunk-major cols
        f = m.shape[1]
        return m.reshape(n, P, f).transpose(1, 0, 2).reshape(P, n * f)

    wpk = np.concatenate(
        [chunked(w1t, NCC), chunked(w2t, NHH), chunked(wat, NCC),
         chunked(wbt, NCC)], axis=1,
    ).astype(bf)

    xs = x.reshape(NCORES, BLOC, C, HW)
    return [{"x": xs[i], "wpk": wpk} for i in range(NCORES)]


_NC_CACHE = None


def _get_nc():
    global _NC_CACHE
    if _NC_CACHE is None:
        _NC_CACHE = build_bass()
    return _NC_CACHE


def run(inputs: dict, trace: bool = False):
    """Run on 8 NeuronCores. Returns (full_output, BassKernelResults)."""
    in_maps = _prep_in_maps(**inputs)
    nc = _get_nc()
    res = run_bass_kernel_spmd(
        nc, in_maps, core_ids=list(range(NCORES)), trace=trace
    )
    out = np.stack([res.results[i]["out"] for i in range(NCORES)])
    return out.reshape(B, C, H, W).astype(np.float32, copy=False), res


def kernel(x, w1, w2, wA, wB):
    out, _ = run({"x": x, "w1": w1, "w2": w2, "wA": wA, "wB": wB})
    return out


# revision 26
# speedup vs baseline: 1.0187x; 1.0187x over previous
"""AdaptiveBlock Trainium2 kernel, 8-core data-parallel.

Reference computation (per batch b):
    y      = mean(x[b], axis=(H, W))                    # (C,)
    h      = gelu(y @ w1.T)                             # (HIDDEN,)
    y'     = gelu(h @ w2.T)                             # (C,)
    A      = (y' @ wA.T).reshape(H, R)
    Bm     = (y' @ wB.T).reshape(R, W)
    attn   = sigmoid(A @ Bm)                            # (H, W)
    out[b] = broadcast attn over C                      # (C, H, W)

Sharding: B=32 split over 8 cores (4 batches/core), weights replicated.
Memory-bound: each core reads 12.8 MB of x and writes 12.8 MB of output;
HBM roofline ~72 us/core.

Device layout / scheduling notes:
- y is computed transposed (C on partitions) directly by free-dim reduces.
- All weights are passed pre-transposed AND pre-cast to bf16 host-side
  (contraction dim on partitions); matmul operands are bf16, PSUM f32.
  The f32 sums feeding the MLP are cast to bf16 on the ScalarEngine.
  All activations here are ~1e-2 magnitude and the output is sigmoid
  (~0.5 +- 1e-4), so bf16 operand noise is ~1e-6 relative on the output.
- wA additionally has its output columns permuted host-side from i*8+r to
  r*56+i so the per-batch (8, 56) bilinear lhsT is a contiguous-row
  SBUF->SBUF DMA.
- The (56, 56) attention map is flattened to (1, 3136) bf16, broadcast to
  128 partitions by a K=1 ones-matmul on the TensorEngine, with sigmoid
  fused into the PSUM->SBUF copy on the ScalarEngine.
- The 4 batches are processed in 2 groups of 2 so group 0's output DMAs
  overlap group 1's input DMAs. Inputs ride the SWDGE (gpsimd) queue,
  outputs + weights the SP HWDGE queue, small SBUF->SBUF moves the ACT
  HWDGE queue - three independent DMA dispatch FIFOs.
"""

import numpy as np
import ml_dtypes

import concourse.bass as bass
import concourse.tile as tile
from concourse import bacc, mybir
from concourse.bass_utils import run_bass_kernel_spmd

F32 = mybir.dt.float32
BF16 = mybir.dt.bfloat16

B, C, H, W = 32, 256, 56, 56
HW = H * W                      # 3136
HIDDEN = 512
RANK = 8
HR = H * RANK                   # 448
NCORES = 8
BLOC = B // NCORES              # 4 batches per core
P = 128
NCC = C // P                    # 2 channel chunks
NHH = HIDDEN // P               # 4 hidden chunks
BCHUNK = 512                    # matmul moving free-dim max (PSUM bank)
NGRP = 2                        # batch groups per core
GB = BLOC // NGRP               # batches per group


def build_bass(sim_compat: bool = False) -> bacc.Bacc:
    """sim_compat=True swaps exact Gelu (not implemented in CoreSim) for a
    0.5*x stand-in; with |gelu-input| ~ 0.02 this perturbs the final sigmoid
    output by ~1e-5 relative, so the sim still validates all layout/dataflow.
    Hardware builds always use the exact erf-based Gelu."""
    gelu_f = (
        mybir.ActivationFunctionType.Copy
        if sim_compat
        else mybir.ActivationFunctionType.Gelu
    )
    gelu_s = 0.5 if sim_compat else 1.0
    nc = bacc.Bacc(num_devices=NCORES)

    HRP = 2 * C                                        # 512: r*64+i padded cols
    WPACK = NCC * HIDDEN + NHH * C + 2 * NCC * HRP     # 4096 bf16 columns
    x_d = nc.dram_tensor("x", [BLOC, C, HW], F32, kind="ExternalInput")
    wpk_d = nc.dram_tensor("wpk", [P, WPACK], BF16, kind="ExternalInput")
    out_d = nc.dram_tensor("out", [BLOC, C, HW], F32, kind="ExternalOutput")

    x_v = x_d.ap().rearrange("b (cc p) hw -> b cc p hw", p=P)
    x_vb = x_d.ap().rearrange("b (cc p) hw -> b p cc hw", p=P)
    out_v = out_d.ap().rearrange("b (cc p) hw -> b cc p hw", p=P)
    OF_W1, OF_W2, OF_WA, OF_WB = 0, NCC * HIDDEN, NCC * HIDDEN + NHH * C, NCC * HIDDEN + NHH * C + NCC * HRP

    nchunks = (HW + BCHUNK - 1) // BCHUNK

    with tile.TileContext(nc) as tc:
        with (
            tc.tile_pool(name="xin", bufs=8) as xpool,
            tc.tile_pool(name="persist", bufs=1) as ppool,
            tc.tile_pool(name="small", bufs=2) as spool,
            tc.tile_pool(name="bc", bufs=4) as bcpool,
            tc.tile_pool(name="ps_small", bufs=2, space="PSUM") as ps_small,
            tc.tile_pool(name="ps_ab", bufs=1, space="PSUM") as ps_ab,
            tc.tile_pool(name="ps_bc", bufs=4, space="PSUM") as ps_bc,
        ):
            # --- single packed weight load; queued in the SWDGE FIFO after
            # the first 4 x chunks (weights are first needed ~6 chunks in,
            # and x chunk 1 gates the whole reduce pipeline)
            wpk = ppool.tile([P, WPACK], BF16, tag="wpk", name="wpk")
            w1t = [wpk[:, OF_W1 + cc * HIDDEN : OF_W1 + (cc + 1) * HIDDEN]
                   for cc in range(NCC)]
            w2t = [wpk[:, OF_W2 + hh * C : OF_W2 + (hh + 1) * C]
                   for hh in range(NHH)]
            wat = [wpk[:, OF_WA + cc * HRP : OF_WA + (cc + 1) * HRP]
                   for cc in range(NCC)]
            wbt = [wpk[:, OF_WB + cc * HRP : OF_WB + (cc + 1) * HRP]
                   for cc in range(NCC)]
            ones = ppool.tile([1, P], BF16, tag="ones", name="ones")
            nc.vector.memset(ones[:], 1.0)

            # --- stream ALL x in up front; spatial-sum per (b, chunk).
            # Group 0's cc=1 sums run on the ScalarEngine (activation
            # accum_out) so the DVE reduce chain isn't the serial
            # bottleneck; everything later in the ACT FIFO belongs to
            # group 0's compute chain, so ACT never head-of-line blocks
            # group 1 (whose sums all run on DVE).
            ysum = [[ppool.tile([P, GB], F32, tag=f"ysum{g}{cc}",
                                name=f"ysum{g}{cc}") for cc in range(NCC)]
                    for g in range(NGRP)]

            def spatial_sum(view, g, cc, j):
                # group 0's cc=1 sums run on ACT (accum_out) in parallel
                # with DVE; everything else on DVE
                if g == 0 and cc == 1:
                    nc.scalar.activation(
                        view, view, mybir.ActivationFunctionType.Copy,
                        accum_out=ysum[g][cc][:, j : j + 1],
                    )
                else:
                    nc.vector.reduce_sum(
                        ysum[g][cc][:, j : j + 1], view,
                        axis=mybir.AxisListType.X,
                    )

            hparts = ppool.tile([P, 2], F32, tag="hparts", name="hparts")
            for b in range(BLOC):
                g, j = divmod(b, GB)
                for cc in range(NCC):
                    xt = xpool.tile([P, HW], F32, tag="xt", name="xt")
                    if b == BLOC - 1 and cc == NCC - 1:
                        # split the last chunk so its reduce tail is half
                        # as long after the stream ends
                        HH = HW // 2
                        nc.gpsimd.dma_start(xt[:, 0:HH], x_v[b, cc][:, 0:HH])
                        nc.vector.reduce_sum(hparts[:, 0:1], xt[:, 0:HH],
                                             axis=mybir.AxisListType.X)
                        nc.gpsimd.dma_start(xt[:, HH:HW], x_v[b, cc][:, HH:HW])
                        nc.vector.reduce_sum(hparts[:, 1:2], xt[:, HH:HW],
                                             axis=mybir.AxisListType.X)
                        nc.vector.tensor_add(
                            ysum[g][cc][:, j : j + 1], hparts[:, 0:1],
                            hparts[:, 1:2],
                        )
                    else:
                        nc.gpsimd.dma_start(xt[:], x_v[b, cc])
                        spatial_sum(xt[:], g, cc, j)
                    if (b, cc) == (1, 1):
                        nc.gpsimd.dma_start(wpk[:], wpk_d.ap())

            def make_ysb(g):
                ysb = [ppool.tile([P, GB], BF16, tag=f"ysb{g}{cc}",
                                  name=f"ysb{g}{cc}") for cc in range(NCC)]
                for cc in range(NCC):
                    if g == 0:
                        nc.scalar.copy(ysb[cc][:], ysum[g][cc][:])
                    else:
                        nc.vector.tensor_copy(ysb[cc][:], ysum[g][cc][:])
                return ysb

            def mlp_group(g, ysb):
                """MLP + A|B matmuls for one batch group -> ab_sb tile."""
                hT = [ppool.tile([P, GB], BF16, tag=f"hT{g}{hh}",
                                 name=f"hT{g}{hh}") for hh in range(NHH)]
                for hh in range(NHH):
                    ph = ps_small.tile([P, GB], F32, tag="ps", name="ps")
                    for cc in range(NCC):
                        nc.tensor.matmul(
                            ph[:], w1t[cc][:, hh * P : (hh + 1) * P], ysb[cc][:],
                            start=(cc == 0), stop=(cc == NCC - 1),
                        )
                    nc.scalar.activation(hT[hh][:], ph[:], gelu_f,
                                         scale=gelu_s / HW)
                ypT = [ppool.tile([P, GB], BF16, tag=f"ypT{g}{cc}",
                                  name=f"ypT{g}{cc}") for cc in range(NCC)]
                for cc in range(NCC):
                    pyp = ps_small.tile([P, GB], F32, tag="ps", name="ps")
                    for hh in range(NHH):
                        nc.tensor.matmul(
                            pyp[:], w2t[hh][:, cc * P : (cc + 1) * P], hT[hh][:],
                            start=(hh == 0), stop=(hh == NHH - 1),
                        )
                    nc.scalar.activation(ypT[cc][:], pyp[:], gelu_f, scale=gelu_s)
                pab = ps_ab.tile([GB, 2 * BCHUNK], F32, tag="pab", name="pab")
                for half, wt in ((0, wat), (1, wbt)):
                    for cc in range(NCC):
                        nc.tensor.matmul(
                            pab[:, half * BCHUNK : (half + 1) * BCHUNK],
                            ypT[cc][:], wt[cc][:],
                            start=(cc == 0), stop=(cc == NCC - 1),
                        )
                ab_sb = ppool.tile([GB, 2 * BCHUNK], BF16, tag=f"ab{g}",
                                   name=f"ab{g}")
                nc.scalar.copy(ab_sb[:], pab[:])
                return ab_sb

            def bilinear_flat(ab_sb, j):
                """Per batch: t = tanh((A @ B)/2) flattened to (1, HW) bf16."""
                ar = spool.tile([RANK, 64], BF16, tag="ar", name="ar")
                br = spool.tile([RANK, 64], BF16, tag="br", name="br")
                nc.sync.dma_start(
                    ar[:],
                    ab_sb[j : j + 1, 0:BCHUNK].rearrange(
                        "o (r i) -> o r i", r=RANK),
                )
                nc.sync.dma_start(
                    br[:],
                    ab_sb[j : j + 1, BCHUNK : 2 * BCHUNK].rearrange(
                        "o (r i) -> o r i", r=RANK),
                )
                pm = ps_small.tile([H, W], F32, tag="ps", name="ps")
                nc.tensor.matmul(pm[:], ar[:, 0:H], br[:, 0:W],
                                 start=True, stop=True)
                msb = spool.tile([H, W], BF16, tag="msb", name="msb")
                nc.scalar.activation(msb[:], pm[:],
                                     mybir.ActivationFunctionType.Tanh,
                                     scale=0.5)
                flat = spool.tile([1, HW], BF16, tag="flat", name="flat")
                nc.sync.dma_start(
                    flat[0:1, :].rearrange("o (i j) -> o i j", i=H), msb[:, :]
                )
                return flat

            def bcast_out(b, flat):
                """Broadcast t to 128 partitions, apply 0.5*t+0.5, store."""
                bc = bcpool.tile([P, HW], F32, tag="bc", name="bc")
                for k in range(nchunks):
                    off = k * BCHUNK
                    sz = min(BCHUNK, HW - off)
                    pb = ps_bc.tile([P, sz], F32, tag="psbc", name="psbc")
                    nc.tensor.matmul(
                        pb[:], ones[:], flat[0:1, off : off + sz],
                        start=True, stop=True,
                    )
                    nc.scalar.activation(
                        bc[:, off : off + sz], pb[:],
                        mybir.ActivationFunctionType.Copy,
                        bias=0.5, scale=0.5,
                    )
                for cc in range(NCC):
                    nc.sync.dma_start(out_v[b, cc], bc[:])

            # Emission (~execution) order: interleave group 1's MLP between
            # batch 0's and batch 1's broadcast sections so the PE FIFO
            # doesn't serialize g1's chain behind both g0 broadcasts.
            ysb0 = make_ysb(0)
            ab0 = mlp_group(0, ysb0)
            flat0 = bilinear_flat(ab0, 0)
            flat1 = bilinear_flat(ab0, 1)
            bcast_out(0, flat0)
            bcast_out(1, flat1)
            ysb1 = make_ysb(1)
            ab1 = mlp_group(1, ysb1)
            flat2 = bilinear_flat(ab1, 0)
            flat3 = bilinear_flat(ab1, 1)
            bcast_out(2, flat2)
            bcast_out(3, flat3)

    nc.compile()
    return nc


def _prep_in_maps(x, w1, w2, wA, wB):
    x = np.ascontiguousarray(np.asarray(x, dtype=np.float32))
    w1 = np.asarray(w1, dtype=np.float32)
    w2 = np.asarray(w2, dtype=np.float32)
    wA = np.asarray(wA, dtype=np.float32)
    wB = np.asarray(wB, dtype=np.float32)

    bf = ml_dtypes.bfloat16
    w1t = np.ascontiguousarray(w1.T)                       # (C, HIDDEN)
    w2t = np.ascontiguousarray(w2.T)                       # (HIDDEN, C)
    # permute wA rows i*8+r -> r*64+i (8 zero pad cols per r) and wB rows
    # r*56+j -> r*64+j, then transpose: 64-aligned r-chunks make the
    # per-batch (8, 56) bilinear operand extraction a clean strided DMA
    HRP = 2 * C
    wap = np.zeros((RANK, 64, C), np.float32)
    wap[:, :H, :] = wA.reshape(H, RANK, C).transpose(1, 0, 2)
    wat = np.ascontiguousarray(wap.reshape(HRP, C).T)
    wbp = np.zeros((RANK, 64, C), np.float32)
    wbp[:, :W, :] = wB.reshape(RANK, W, C)
    wbt = np.ascontiguousarray(wbp.reshape(HRP, C).T)

    # pack per-partition: [w1t cc-chunks | w2t hh-chunks | wat | wbt]
    def chunked(m, n):          # (n*128, F) -> (128, n*F), chunk-major cols
        f = m.shape[1]
        return m.reshape(n, P, f).transpose(1, 0, 2).reshape(P, n * f)

    wpk = np.concatenate(
        [chunked(w1t, NCC), chunked(w2t, NHH), chunked(wat, NCC),
         chunked(wbt, NCC)], axis=1,
    ).astype(bf)

    xs = x.reshape(NCORES, BLOC, C, HW)
    return [{"x": xs[i], "wpk": wpk} for i in range(NCORES)]


_NC_CACHE = None


def _get_nc():
    global _NC_CACHE
    if _NC_CACHE is None:
        _NC_CACHE = build_bass()
    return _NC_CACHE


def run(inputs: dict, trace: bool = False):
    """Run on 8 NeuronCores. Returns (full_output, BassKernelResults)."""
    in_maps = _prep_in_maps(**inputs)
    nc = _get_nc()
    res = run_bass_kernel_spmd(
        nc, in_maps, core_ids=list(range(NCORES)), trace=trace
    )
    out = np.stack([res.results[i]["out"] for i in range(NCORES)])
    return out.reshape(B, C, H, W).astype(np.float32, copy=False), res


def kernel(x, w1, w2, wA, wB):
    out, _ = run({"x": x, "w1": w1, "w2": w2, "wA": wA, "wB": wB})
    return out


# revision 27
# speedup vs baseline: 1.0769x; 1.0571x over previous
"""AdaptiveBlock Trainium2 kernel, 8-core data-parallel.

Reference computation (per batch b):
    y      = mean(x[b], axis=(H, W))                    # (C,)
    h      = gelu(y @ w1.T)                             # (HIDDEN,)
    y'     = gelu(h @ w2.T)                             # (C,)
    A      = (y' @ wA.T).reshape(H, R)
    Bm     = (y' @ wB.T).reshape(R, W)
    attn   = sigmoid(A @ Bm)                            # (H, W)
    out[b] = broadcast attn over C                      # (C, H, W)

Sharding: B=32 split over 8 cores (4 batches/core), weights replicated.
Memory-bound: each core reads 12.8 MB of x and writes 12.8 MB of output;
HBM roofline ~72 us/core.

Device layout / scheduling notes:
- y is computed transposed (C on partitions) directly by free-dim reduces.
- All weights are passed pre-transposed AND pre-cast to bf16 host-side
  (contraction dim on partitions); matmul operands are bf16, PSUM f32.
  The f32 sums feeding the MLP are cast to bf16 on the ScalarEngine.
  All activations here are ~1e-2 magnitude and the output is sigmoid
  (~0.5 +- 1e-4), so bf16 operand noise is ~1e-6 relative on the output.
- wA additionally has its output columns permuted host-side from i*8+r to
  r*56+i so the per-batch (8, 56) bilinear lhsT is a contiguous-row
  SBUF->SBUF DMA.
- The (56, 56) attention map is flattened to (1, 3136) bf16, broadcast to
  128 partitions by a K=1 ones-matmul on the TensorEngine, with sigmoid
  fused into the PSUM->SBUF copy on the ScalarEngine.
- The 4 batches are processed in 2 groups of 2 so group 0's output DMAs
  overlap group 1's input DMAs. Inputs ride the SWDGE (gpsimd) queue,
  outputs + weights the SP HWDGE queue, small SBUF->SBUF moves the ACT
  HWDGE queue - three independent DMA dispatch FIFOs.
"""

import numpy as np
import ml_dtypes

import concourse.bass as bass
import concourse.tile as tile
from concourse import bacc, mybir
from concourse.bass_utils import run_bass_kernel_spmd

F32 = mybir.dt.float32
BF16 = mybir.dt.bfloat16

B, C, H, W = 32, 256, 56, 56
HW = H * W                      # 3136
HIDDEN = 512
RANK = 8
HR = H * RANK                   # 448
NCORES = 8
BLOC = B // NCORES              # 4 batches per core
P = 128
NCC = C // P                    # 2 channel chunks
NHH = HIDDEN // P               # 4 hidden chunks
BCHUNK = 512                    # matmul moving free-dim max (PSUM bank)
NGRP = 2                        # batch groups per core
GB = BLOC // NGRP               # batches per group


def build_bass(sim_compat: bool = False) -> bacc.Bacc:
    """sim_compat=True swaps exact Gelu (not implemented in CoreSim) for a
    0.5*x stand-in; with |gelu-input| ~ 0.02 this perturbs the final sigmoid
    output by ~1e-5 relative, so the sim still validates all layout/dataflow.
    Hardware builds always use the exact erf-based Gelu."""
    gelu_f = (
        mybir.ActivationFunctionType.Copy
        if sim_compat
        else mybir.ActivationFunctionType.Gelu
    )
    gelu_s = 0.5 if sim_compat else 1.0
    nc = bacc.Bacc(num_devices=NCORES)

    HRP = 2 * C                                        # 512: r*64+i padded cols
    WPACK = NCC * HIDDEN + NHH * C + 2 * NCC * HRP     # 4096 bf16 columns
    x_d = nc.dram_tensor("x", [BLOC, C, HW], F32, kind="ExternalInput")
    wpk_d = nc.dram_tensor("wpk", [P, WPACK], BF16, kind="ExternalInput")
    out_d = nc.dram_tensor("out", [BLOC, C, HW], F32, kind="ExternalOutput")

    x_v = x_d.ap().rearrange("b (cc p) hw -> b cc p hw", p=P)
    x_vb = x_d.ap().rearrange("b (cc p) hw -> b p cc hw", p=P)
    out_v = out_d.ap().rearrange("b (cc p) hw -> b cc p hw", p=P)
    OF_W1, OF_W2, OF_WA, OF_WB = 0, NCC * HIDDEN, NCC * HIDDEN + NHH * C, NCC * HIDDEN + NHH * C + NCC * HRP

    nchunks = (HW + BCHUNK - 1) // BCHUNK

    with tile.TileContext(nc) as tc:
        with (
            tc.tile_pool(name="xin", bufs=8) as xpool,
            tc.tile_pool(name="persist", bufs=1) as ppool,
            tc.tile_pool(name="small", bufs=2) as spool,
            tc.tile_pool(name="bc", bufs=4) as bcpool,
            tc.tile_pool(name="ps_small", bufs=2, space="PSUM") as ps_small,
            tc.tile_pool(name="ps_ab", bufs=1, space="PSUM") as ps_ab,
            tc.tile_pool(name="ps_bc", bufs=4, space="PSUM") as ps_bc,
        ):
            # --- single packed weight load; queued in the SWDGE FIFO after
            # the first 4 x chunks (weights are first needed ~6 chunks in,
            # and x chunk 1 gates the whole reduce pipeline)
            wpk = ppool.tile([P, WPACK], BF16, tag="wpk", name="wpk")
            w1t = [wpk[:, OF_W1 + cc * HIDDEN : OF_W1 + (cc + 1) * HIDDEN]
                   for cc in range(NCC)]
            w2t = [wpk[:, OF_W2 + hh * C : OF_W2 + (hh + 1) * C]
                   for hh in range(NHH)]
            wat = [wpk[:, OF_WA + cc * HRP : OF_WA + (cc + 1) * HRP]
                   for cc in range(NCC)]
            wbt = [wpk[:, OF_WB + cc * HRP : OF_WB + (cc + 1) * HRP]
                   for cc in range(NCC)]
            ones = ppool.tile([1, P], BF16, tag="ones", name="ones")
            nc.vector.memset(ones[:], 1.0)

            # --- stream ALL x in up front; spatial-sum per (b, chunk).
            # Group 0's cc=1 sums run on the ScalarEngine (activation
            # accum_out) so the DVE reduce chain isn't the serial
            # bottleneck; everything later in the ACT FIFO belongs to
            # group 0's compute chain, so ACT never head-of-line blocks
            # group 1 (whose sums all run on DVE).
            ysum = [[ppool.tile([P, GB], F32, tag=f"ysum{g}{cc}",
                                name=f"ysum{g}{cc}") for cc in range(NCC)]
                    for g in range(NGRP)]

            def spatial_sum(view, g, cc, j):
                # group 0's cc=1 sums run on ACT (accum_out) in parallel
                # with DVE; everything else on DVE
                if g == 0 and cc == 1:
                    nc.scalar.activation(
                        view, view, mybir.ActivationFunctionType.Copy,
                        accum_out=ysum[g][cc][:, j : j + 1],
                    )
                else:
                    nc.vector.reduce_sum(
                        ysum[g][cc][:, j : j + 1], view,
                        axis=mybir.AxisListType.X,
                    )

            hparts = ppool.tile([P, 2], F32, tag="hparts", name="hparts")
            for b in range(BLOC):
                g, j = divmod(b, GB)
                for cc in range(NCC):
                    xt = xpool.tile([P, HW], F32, tag="xt", name="xt")
                    if b == BLOC - 1 and cc == NCC - 1:
                        # split the last chunk so its reduce tail is half
                        # as long after the stream ends
                        HH = HW // 2
                        nc.gpsimd.dma_start(xt[:, 0:HH], x_v[b, cc][:, 0:HH])
                        nc.vector.reduce_sum(hparts[:, 0:1], xt[:, 0:HH],
                                             axis=mybir.AxisListType.X)
                        nc.gpsimd.dma_start(xt[:, HH:HW], x_v[b, cc][:, HH:HW])
                        nc.vector.reduce_sum(hparts[:, 1:2], xt[:, HH:HW],
                                             axis=mybir.AxisListType.X)
                        nc.vector.tensor_add(
                            ysum[g][cc][:, j : j + 1], hparts[:, 0:1],
                            hparts[:, 1:2],
                        )
                    else:
                        nc.gpsimd.dma_start(xt[:], x_v[b, cc])
                        spatial_sum(xt[:], g, cc, j)
                    if (b, cc) == (1, 1):
                        nc.gpsimd.dma_start(wpk[:], wpk_d.ap())

            def make_ysb(g):
                ysb = [ppool.tile([P, GB], BF16, tag=f"ysb{g}{cc}",
                                  name=f"ysb{g}{cc}") for cc in range(NCC)]
                for cc in range(NCC):
                    if g == 0:
                        nc.scalar.copy(ysb[cc][:], ysum[g][cc][:])
                    else:
                        nc.vector.tensor_copy(ysb[cc][:], ysum[g][cc][:])
                return ysb

            def mlp_group(g, ysb):
                """MLP + A|B matmuls for one batch group -> ab_sb tile."""
                hT = [ppool.tile([P, GB], BF16, tag=f"hT{g}{hh}",
                                 name=f"hT{g}{hh}") for hh in range(NHH)]
                for hh in range(NHH):
                    ph = ps_small.tile([P, GB], F32, tag="ps", name="ps")
                    for cc in range(NCC):
                        nc.tensor.matmul(
                            ph[:], w1t[cc][:, hh * P : (hh + 1) * P], ysb[cc][:],
                            start=(cc == 0), stop=(cc == NCC - 1),
                        )
                    nc.scalar.activation(hT[hh][:], ph[:], gelu_f,
                                         scale=gelu_s / HW)
                ypT = [ppool.tile([P, GB], BF16, tag=f"ypT{g}{cc}",
                                  name=f"ypT{g}{cc}") for cc in range(NCC)]
                for cc in range(NCC):
                    pyp = ps_small.tile([P, GB], F32, tag="ps", name="ps")
                    for hh in range(NHH):
                        nc.tensor.matmul(
                            pyp[:], w2t[hh][:, cc * P : (cc + 1) * P], hT[hh][:],
                            start=(hh == 0), stop=(hh == NHH - 1),
                        )
                    nc.scalar.activation(ypT[cc][:], pyp[:], gelu_f, scale=gelu_s)
                pab = ps_ab.tile([GB, 2 * BCHUNK], F32, tag="pab", name="pab")
                for half, wt in ((0, wat), (1, wbt)):
                    for cc in range(NCC):
                        nc.tensor.matmul(
                            pab[:, half * BCHUNK : (half + 1) * BCHUNK],
                            ypT[cc][:], wt[cc][:],
                            start=(cc == 0), stop=(cc == NCC - 1),
                        )
                ab_sb = ppool.tile([GB, 2 * BCHUNK], BF16, tag=f"ab{g}",
                                   name=f"ab{g}")
                nc.scalar.copy(ab_sb[:], pab[:])
                return ab_sb

            def bilinear_flat(ab_sb, j):
                """Per batch: t = tanh((A @ B)/2) flattened to (1, HW) bf16."""
                ar = spool.tile([RANK, 64], BF16, tag="ar", name="ar")
                br = spool.tile([RANK, 64], BF16, tag="br", name="br")
                nc.sync.dma_start(
                    ar[:],
                    ab_sb[j : j + 1, 0:BCHUNK].rearrange(
                        "o (r i) -> o r i", r=RANK),
                )
                nc.sync.dma_start(
                    br[:],
                    ab_sb[j : j + 1, BCHUNK : 2 * BCHUNK].rearrange(
                        "o (r i) -> o r i", r=RANK),
                )
                pm = ps_small.tile([H, W], F32, tag="ps", name="ps")
                nc.tensor.matmul(pm[:], ar[:, 0:H], br[:, 0:W],
                                 start=True, stop=True)
                msb = spool.tile([H, W], BF16, tag="msb", name="msb")
                nc.scalar.activation(msb[:], pm[:],
                                     mybir.ActivationFunctionType.Tanh,
                                     scale=0.5)
                flat = spool.tile([1, HW], BF16, tag="flat", name="flat")
                nc.sync.dma_start(
                    flat[0:1, :].rearrange("o (i j) -> o i j", i=H), msb[:, :]
                )
                return flat

            def bcast_out(b, flat):
                """Broadcast t to 128 partitions, apply 0.5*t+0.5, store."""
                bc = bcpool.tile([P, HW], F32, tag="bc", name="bc")
                for k in range(nchunks):
                    off = k * BCHUNK
                    sz = min(BCHUNK, HW - off)
                    pb = ps_bc.tile([P, sz], F32, tag="psbc", name="psbc")
                    nc.tensor.matmul(
                        pb[:], ones[:], flat[0:1, off : off + sz],
                        start=True, stop=True,
                    )
                    # alternate the affine PSUM->SBUF copies between ACT
                    # and DVE so neither engine serializes the batch chain
                    if k % 2 == 0:
                        nc.scalar.activation(
                            bc[:, off : off + sz], pb[:],
                            mybir.ActivationFunctionType.Copy,
                            bias=0.5, scale=0.5,
                        )
                    else:
                        nc.vector.tensor_scalar(
                            bc[:, off : off + sz], pb[:], 0.5, 0.5,
                            mybir.AluOpType.mult, mybir.AluOpType.add,
                        )
                for cc in range(NCC):
                    nc.sync.dma_start(out_v[b, cc], bc[:])

            # Emission (~execution) order: interleave group 1's MLP between
            # batch 0's and batch 1's broadcast sections so the PE FIFO
            # doesn't serialize g1's chain behind both g0 broadcasts.
            ysb0 = make_ysb(0)
            ab0 = mlp_group(0, ysb0)
            flat0 = bilinear_flat(ab0, 0)
            flat1 = bilinear_flat(ab0, 1)
            bcast_out(0, flat0)
            bcast_out(1, flat1)
            ysb1 = make_ysb(1)
            ab1 = mlp_group(1, ysb1)
            flat2 = bilinear_flat(ab1, 0)
            flat3 = bilinear_flat(ab1, 1)
            bcast_out(2, flat2)
            bcast_out(3, flat3)

    nc.compile()
    return nc


def _prep_in_maps(x, w1, w2, wA, wB):
    x = np.ascontiguousarray(np.asarray(x, dtype=np.float32))
    w1 = np.asarray(w1, dtype=np.float32)
    w2 = np.asarray(w2, dtype=np.float32)
    wA = np.asarray(wA, dtype=np.float32)
    wB = np.asarray(wB, dtype=np.float32)

    bf = ml_dtypes.bfloat16
    w1t = np.ascontiguousarray(w1.T)                       # (C, HIDDEN)
    w2t = np.ascontiguousarray(w2.T)                       # (HIDDEN, C)
    # permute wA rows i*8+r -> r*64+i (8 zero pad cols per r) and wB rows
    # r*56+j -> r*64+j, then transpose: 64-aligned r-chunks make the
    # per-batch (8, 56) bilinear operand extraction a clean strided DMA
    HRP = 2 * C
    wap = np.zeros((RANK, 64, C), np.float32)
    wap[:, :H, :] = wA.reshape(H, RANK, C).transpose(1, 0, 2)
    wat = np.ascontiguousarray(wap.reshape(HRP, C).T)
    wbp = np.zeros((RANK, 64, C), np.float32)
    wbp[:, :W, :] = wB.reshape(RANK, W, C)
    wbt = np.ascontiguousarray(wbp.reshape(HRP, C).T)

    # pack per-partition: [w1t cc-chunks | w2t hh-chunks | wat | wbt]
    def chunked(m, n):          # (n*128, F) -> (128, n*F), chunk-major cols
        f = m.shape[1]
        return m.reshape(n, P, f).transpose(1, 0, 2).reshape(P, n * f)

    wpk = np.concatenate(
        [chunked(w1t, NCC), chunked(w2t, NHH), chunked(wat, NCC),
         chunked(wbt, NCC)], axis=1,
    ).astype(bf)

    xs = x.reshape(NCORES, BLOC, C, HW)
    return [{"x": xs[i], "wpk": wpk} for i in range(NCORES)]


_NC_CACHE = None


def _get_nc():
    global _NC_CACHE
    if _NC_CACHE is None:
        _NC_CACHE = build_bass()
    return _NC_CACHE


def run(inputs: dict, trace: bool = False):
    """Run on 8 NeuronCores. Returns (full_output, BassKernelResults)."""
    in_maps = _prep_in_maps(**inputs)
    nc = _get_nc()
    res = run_bass_kernel_spmd(
        nc, in_maps, core_ids=list(range(NCORES)), trace=trace
    )
    out = np.stack([res.results[i]["out"] for i in range(NCORES)])
    return out.reshape(B, C, H, W).astype(np.float32, copy=False), res


def kernel(x, w1, w2, wA, wB):
    out, _ = run({"x": x, "w1": w1, "w2": w2, "wA": wA, "wB": wB})
    return out
